# revision 65
# baseline (speedup 1.0000x reference)
"""Trainium2 Bass kernel for nn_AdaptiveFourierFeatures.

Strategy
--------
The reference module computes, per batch b and token s:

    q[s,h,:]   depends on x[s] through two linear layers
    k[d,f,h,:] = f[d,f]*u[h,:] + v[h,:]         (keys are AFFINE in f[d,f]
                                                  because key_proj is Linear(1,A))
    scores[s,d,h,f] = q.k/sqrt(HD) = alpha[s,h]*f[d,f] + beta[s,h]

With the given inputs, freq_matrix*freq_scale has IDENTICAL rows
(f[d,:] == g[:] for all d), so softmax over f is d-independent and beta
cancels inside the softmax:

    attn[s,h,f] = softmax_f(alpha[s,h] * (g[f]-gc))      (gc: shift for range)
    aw[s,f]     = mean_h attn[s,h,f]

The fourier features contract with the MLP weights analytically using
sin(theta+phi) = sin*cos + cos*sin, folding phase and the D dimension into
small [F,O] matrices on the host.  The device pipeline per token is then:

    x(64) -> alpha-scores(64=H*F) -> softmax -> aw features z(32)
          -> [x|z|1](97) @ G(97x128) -> sigmoid*silu gate -> residual

Sharding: data-parallel over batch B=8, one batch element per NeuronCore.
All folded parameters are tiny and replicated.

kernel(**inputs) takes the FULL inputs and returns the FULL [B,S,D] output.
"""

import sys

import numpy as np
import ml_dtypes

# concourse (bass) lives in the trn repo; make sure it is importable even if
# the harness runs from a directory without the site defaults.
for _p in ("/opt/trn_rl_repo", "/opt/pypackages"):
    if _p not in sys.path:
        sys.path.append(_p)

# ---- problem constants (hardcoded; kernel.py must be self-contained) ----
B, S, D, F, A, H, O = 8, 2048, 64, 16, 32, 4, 64
HD = A // H
TWO_PI = 2.0 * np.pi
N_CORES = 8
HF = H * F            # 64 score columns per token
NFEAT = D + 2 * F + 1  # 97 = x | z_sin | z_cos | ones
HALF = S // 2          # stacked-half layout: 1024 tokens per half

BF16 = ml_dtypes.bfloat16
NCHUNKS = 2

_CACHE = {}


def _make_xT(xb: np.ndarray) -> np.ndarray:
    """[S, D] batch slice -> chunk-major transposed bf16 [D, S] layout."""
    xt = xb.T
    cw = HALF // NCHUNKS
    pieces = []
    for c in range(NCHUNKS):
        pieces.append(xt[:, c * cw:(c + 1) * cw])
        pieces.append(xt[:, HALF + c * cw:HALF + (c + 1) * cw])
    return np.ascontiguousarray(np.concatenate(pieces, axis=1)).astype(BF16)


def _build_program(nchunks: int = NCHUNKS, enable_asserts: bool = True,
                   enable_partition_id: bool = True,
                   ndum: int = 18, dumn: int = 128):
    """Build the 8-core SPMD bass program (per-core shapes)."""
    import concourse.bass as bass
    import concourse.bacc as bacc
    import concourse.tile as tile
    from concourse import mybir
    from bass_rust import add_dep_helper

    dt = mybir.dt
    AF = mybir.ActivationFunctionType
    ALU = mybir.AluOpType

    nc = bacc.Bacc("TRN2", target_bir_lowering=False, debug=False,
                   enable_asserts=enable_asserts, num_devices=N_CORES,
                   enable_partition_id=enable_partition_id)

    # ---- per-core DRAM parameters ----
    # all bf16 params packed into one [128, 361] array:
    #   wsc [64,64] @cols 0:64, o1 [128,8] @64:72, e2q [8,128] @72:200,
    #   o2 [128,32] @200:232, G [97,128] @232:360, b_score [128,1] @360
    xT = nc.dram_tensor("xT", [D, S], dt.bfloat16, kind="ExternalInput").ap()
    xn = nc.dram_tensor("xn", [S, D], dt.float32, kind="ExternalInput").ap()
    trig = nc.dram_tensor("trig", [2 * F, S], dt.bfloat16, kind="ExternalInput").ap()
    pk = nc.dram_tensor("pk", [128, 361], dt.bfloat16, kind="ExternalInput").ap()
    out_d = nc.dram_tensor("out", [S, D], dt.float32, kind="ExternalOutput").ap()

    KT = S // 128                # 16 token tiles of 128
    CW = HALF // nchunks         # stacked-column chunk width

    with tile.TileContext(nc) as tc:
        with (
            tc.tile_pool(name="const", bufs=1) as cpool,
            tc.tile_pool(name="sb", bufs=1) as sb,
            tc.tile_pool(name="work", bufs=3) as wk,
            tc.tile_pool(name="ps2", bufs=2, space="PSUM") as ps2,
            tc.tile_pool(name="ps", bufs=1, space="PSUM") as ps,
            tc.tile_pool(name="ps_big", bufs=1, space="PSUM") as psb,
        ):
            # ---- inputs to SBUF (x first — it gates compute; two HWDGE rings) ----
            # CZ = [x^T (0:64) | zs (64:80) | zc (80:96) | ones (96)]
            # xT arrives chunk-major from the host: chunk c occupies source
            # columns [c*2CW, (c+1)*2CW) = tokens {c*CW..} U {HALF+c*CW..},
            # so chunk 0's scores can start after the first piece lands.
            cz = sb.tile([NFEAT, S], dt.bfloat16)
            czx_v = cz[0:D, :].rearrange("d (h c) -> d h c", h=2)
            for c in range(nchunks):
                lo = c * CW
                nc.sync.dma_start(out=czx_v[:, :, lo:lo + CW],
                                  in_=xT[:, c * 2 * CW:(c + 1) * 2 * CW])

            c_pk = cpool.tile([128, 361], dt.bfloat16)
            nc.scalar.dma_start(out=c_pk[:], in_=pk[:])
            c_wsc = c_pk[0:D, 0:64]
            c_o1 = c_pk[0:128, 64:72]
            c_e2q = c_pk[0:8, 72:200]
            c_o2 = c_pk[0:128, 200:232]
            c_g = c_pk[0:NFEAT, 232:360]

            # natural-layout x for the residual: [128, (k,64)].  Held back
            # until chunk 0's exp so its 512KB doesn't compete with the
            # critical x^T pieces on the SDMA engines (first use is ~19us).
            xn_t = sb.tile([128, KT * D], dt.float32)
            xn_dma = nc.scalar.dma_start(
                out=xn_t[:],
                in_=xn.rearrange("(k p) d -> p k d", p=128),
            )

            # trig table lives on partitions 64..95 to lane-align with CZ
            # (also held back past chunk 0's exp; first use ~15.6us)
            c_trig = cpool.tile([96, S], dt.bfloat16)
            trig_dma = nc.sync.dma_start(out=c_trig[64:96, :], in_=trig[:])

            # exp bias column (fp32 for the activation bias operand)
            c_bsc = cpool.tile([128, 1], dt.float32)
            nc.vector.tensor_copy(c_bsc[:], c_pk[:, 360:361])

            nc.vector.memset(cz[NFEAT - 1:NFEAT, :], 1.0)

            # warm up the activation table set (exp/tanh share one set)
            warm = cpool.tile([1, 2], dt.float32)
            nc.vector.memset(warm[:], 0.0)
            nc.scalar.activation(warm[:], warm[:], AF.Exp)

            # f32r copy of the 1/den-broadcast matrix for the f32r matmul
            e2f = cpool.tile([8, 128], dt.float32r)
            nc.vector.tensor_copy(e2f[:], c_e2q)

            # PE warm-up: full-array matmuls during the input-DMA wait flip
            # the HAM clock gate to 2.4 GHz before real work arrives.  The
            # operand tile is never written — garbage values are fine, the
            # psum slot is reused (and start=True-cleared) by later work.
            wgarb = cpool.tile([128, 512], dt.bfloat16)
            nc.gpsimd.memset(wgarb[:], 0.0)
            pdum = ps.tile([128, 512], dt.float32, tag="den")
            for _ in range(ndum):
                nc.tensor.matmul(pdum[:, 0:dumn], wgarb[:, 0:128],
                                 wgarb[:, 0:dumn], tile_position=(0, 0))

            for c in range(nchunks):
                lo = c * CW                      # stacked column offset
                # token ranges covered by this chunk (one per half)
                tok_los = (lo, HALF + lo)

                # -- scores: S2[half*64+hf, col] = sum_d x^T[d, tok] Wsc[d, hf]
                s2 = ps2.tile([128, CW], dt.float32, tag="s2")
                for h in range(2):
                    t0 = tok_los[h]
                    for n0 in range(0, CW, 512):
                        nn = min(512, CW - n0)
                        nc.tensor.matmul(
                            s2[h * 64:(h + 1) * 64, n0:n0 + nn],
                            c_wsc,
                            cz[0:D, t0 + n0:t0 + n0 + nn],
                            tile_position=(0, h * 64),
                        )

                # -- exp (bias adds the constant alpha-offset term)
                e1 = wk.tile([128, CW], dt.bfloat16, tag="e1")
                exp_inst = nc.scalar.activation(e1[:], s2[:], AF.Exp,
                                                bias=c_bsc[:])
                if c == 0:
                    add_dep_helper(xn_dma.ins, exp_inst.ins, sync=True,
                                   reason="delay xn load past critical xT")

                # -- denominators: den[(half,h), col] = sum_f e1
                den = ps.tile([8, CW], dt.float32, tag="den")
                for n0 in range(0, CW, 512):
                    nn = min(512, CW - n0)
                    nc.tensor.matmul(
                        den[:, n0:n0 + nn], c_o1, e1[:, n0:n0 + nn],
                        tile_position=(0, 0),
                    )

                # -- reciprocal (fast Newton approx, ~18 bits), f32r-rounded
                from concourse.dve_ops import (
                    RECIP_APPROX_FAST_CONSTS as _RC,
                    RECIPROCAL_APPROX_FAST as _RAF,
                )
                rec = wk.tile([8, CW], dt.float32r, tag="rec")
                nc.vector._custom_dve(_RAF, out=rec[:], in0=den[:],
                                      s0=_RC["s0"], s1=_RC["s1"], imm2=_RC["imm2"])

                # -- broadcast 1/den back to all 128 rows (x0.25 head-mean);
                # f32r streams at bf16 rate for N>=256, no bf16 cast needed
                rb = ps.tile([128, CW], dt.float32, tag="rb")
                for n0 in range(0, CW, 512):
                    nn = min(512, CW - n0)
                    nc.tensor.matmul(
                        rb[:, n0:n0 + nn],
                        e2f[:],
                        rec[:, n0:n0 + nn],
                        tile_position=(0, 0),
                    )

                # -- attn/4 = e1 * rb
                at = wk.tile([128, CW], dt.bfloat16, tag="at")
                nc.vector.tensor_mul(at[:], e1[:], rb[:])

                # -- aw rows (duplicated for sin/cos) on partitions 64..95
                aw = psb.tile([96, 2 * CW], dt.float32, tag="aw")
                for h in range(2):
                    for n0 in range(0, CW, 512):
                        nn = min(512, CW - n0)
                        nc.tensor.matmul(
                            aw[64:96, h * CW + n0:h * CW + n0 + nn],
                            c_o2[h * 64:(h + 1) * 64, :],
                            at[h * 64:(h + 1) * 64, n0:n0 + nn],
                            tile_position=(h * 64, 64),
                        )

                # -- per-half tail: z -> MLP -> tanh -> gate -> residual -> out
                kpc = CW // 128  # k-tiles per token range
                out_v = out_d.rearrange("(k p) d -> p k d", p=128)
                for h in range(2):
                    t0 = tok_los[h]
                    k0 = t0 // 128
                    # z features into CZ rows 64..96 (aw * sin/cos table)
                    nc.vector.tensor_mul(
                        cz[64:96, t0:t0 + CW],
                        aw[64:96, h * CW:(h + 1) * CW],
                        c_trig[64:96, t0:t0 + CW],
                    )
                    # MLP: per 128-token tile, pre = CZ_tile^T @ G (nat layout)
                    pre = psb.tile([128, kpc * 128], dt.float32, tag=f"pre{h}")
                    for i in range(kpc):
                        nc.tensor.matmul(
                            pre[:, i * 128:(i + 1) * 128],
                            cz[:, (k0 + i) * 128:(k0 + i + 1) * 128],
                            c_g,
                            tile_position=(0, 0),
                        )
                    pre_v = pre[:].rearrange("p (j o) -> p j o", j=kpc)
                    th = wk.tile([128, kpc * 128], dt.bfloat16, tag=f"th{h}")
                    th_v = th[:].rearrange("p (j o) -> p j o", j=kpc)
                    wt = wk.tile([128, kpc * 64], dt.bfloat16, tag=f"wt{h}")
                    wt_v = wt[:].rearrange("p (j o) -> p j o", j=kpc)
                    gt = wk.tile([128, kpc * 64], dt.bfloat16, tag=f"gt{h}")
                    out_t = wk.tile([128, kpc * 64], dt.float32, tag=f"outc{h}")
                    # tanh(pre/2); sigmoid(a)=0.5+0.5*tanh(a/2)
                    nc.scalar.activation(th[:], pre[:], AF.Tanh, scale=0.5)
                    # w = (1+tanh_p) * pre_p   [silu*2]
                    nc.vector.scalar_tensor_tensor(
                        wt_v, th_v[:, :, 64:128], 1.0, pre_v[:, :, 64:128],
                        ALU.add, ALU.mult,
                    )
                    # gated*4 = (1+tanh_g) * w
                    nc.vector.scalar_tensor_tensor(
                        gt[:], th_v[:, :, 0:64], 1.0, wt_v,
                        ALU.add, ALU.mult,
                    )
                    # out = gated*0.25 + x, then DMA out (HWDGE)
                    nc.vector.scalar_tensor_tensor(
                        out_t[:], gt[:], 0.25,
                        xn_t[:, k0 * 64:(k0 + kpc) * 64],
                        ALU.mult, ALU.add,
                    )
                    eng = nc.sync if h == 0 else nc.scalar
                    eng.dma_start(out=out_v[:, k0:k0 + kpc, :], in_=out_t[:])

    nc.compile()
    return nc


def _fold_params(inputs):
    """Host-side constant folding (float64).  Returns per-core arrays."""
    f = (np.asarray(inputs["freq_matrix"], np.float64)
         * np.asarray(inputs["freq_scale"], np.float64))
    g = f[0]
    gc = 0.5 * (g.max() + g.min())
    gsh = g - gc

    Wq = np.asarray(inputs["Wq"], np.float64)
    bq = np.asarray(inputs["bq"], np.float64)
    Wk1 = np.asarray(inputs["Wk1"], np.float64)
    bk1 = np.asarray(inputs["bk1"], np.float64)
    Wqi = np.asarray(inputs["Wqi"], np.float64)
    bqi = np.asarray(inputs["bqi"], np.float64)
    Wki = np.asarray(inputs["Wki"], np.float64)
    bki = np.asarray(inputs["bki"], np.float64)
    ph = np.asarray(inputs["phase"], np.float64)

    u = Wki @ Wk1[:, 0]
    Wqq = Wqi @ Wq
    bqq = Wqi @ bq + bqi
    u_h = u.reshape(H, HD)
    M_alpha = np.einsum("he,hed->hd", u_h, Wqq.reshape(H, HD, D)) / np.sqrt(HD)
    c_alpha = np.einsum("he,he->h", u_h, bqq.reshape(H, HD)) / np.sqrt(HD)

    W_score = np.einsum("hd,f->dhf", M_alpha, gsh).reshape(D, HF)
    b_score = np.einsum("h,f->hf", c_alpha, gsh).reshape(HF)
    b_score2 = np.concatenate([b_score, b_score]).reshape(128, 1)

    t = np.linspace(0.0, 1.0, S)
    theta = TWO_PI * t[:, None] * g[None, :]
    trig = np.concatenate([np.sin(theta).T, np.cos(theta).T], 0)  # [2F, S]

    cph, sph = np.cos(ph), np.sin(ph)

    def fold_mlp(W):
        W = np.asarray(W, np.float64)
        Wx = W[:, :D]
        Wf = W[:, D:].reshape(O, D, 2 * F)
        Ws, Wc = Wf[:, :, :F], Wf[:, :, F:]
        Us = np.einsum("df,odf->fo", cph, Ws) - np.einsum("df,odf->fo", sph, Wc)
        Uc = np.einsum("df,odf->fo", sph, Ws) + np.einsum("df,odf->fo", cph, Wc)
        return Wx, Us, Uc

    Wgx, Ugs, Ugc = fold_mlp(inputs["Wg"])
    Wpx, Ups, Upc = fold_mlp(inputs["Wp"])
    bg = np.asarray(inputs["bg"], np.float64)
    bp = np.asarray(inputs["bp"], np.float64)

    G = np.zeros((NFEAT, 128))
    G[0:D, 0:64] = Wgx.T
    G[D:D + F, 0:64] = Ugs
    G[D + F:D + 2 * F, 0:64] = Ugc
    G[NFEAT - 1, 0:64] = bg
    G[0:D, 64:128] = Wpx.T
    G[D:D + F, 64:128] = Ups
    G[D + F:D + 2 * F, 64:128] = Upc
    G[NFEAT - 1, 64:128] = bp

    # indicator matrices for the softmax plumbing
    p = np.arange(128)
    O1 = (p[:, None] // 16 == np.arange(8)[None, :]).astype(np.float64)
    E2q = 0.25 * (np.arange(8)[:, None] == p[None, :] // 16).astype(np.float64)
    O2 = ((p[:, None] % 16) == (np.arange(32)[None, :] % 16)).astype(np.float64)

    # pack all bf16 params into one [128, 361] array (see _build_program)
    pk = np.zeros((128, 361))
    pk[0:D, 0:64] = W_score
    pk[0:128, 64:72] = O1
    pk[0:8, 72:200] = E2q
    pk[0:128, 200:232] = O2
    pk[0:NFEAT, 232:360] = G
    pk[:, 360] = b_score2[:, 0]

    return dict(
        trig=trig.astype(BF16),
        pk=pk.astype(BF16),
    ), gsh, M_alpha, c_alpha


def _numpy_fallback(inputs):
    """Exact collapsed computation in numpy (general freq rows)."""
    x = np.asarray(inputs["x"], np.float64)
    f = (np.asarray(inputs["freq_matrix"], np.float64)
         * np.asarray(inputs["freq_scale"], np.float64))
    Wq = np.asarray(inputs["Wq"], np.float64); bq = np.asarray(inputs["bq"], np.float64)
    Wk1 = np.asarray(inputs["Wk1"], np.float64); bk1 = np.asarray(inputs["bk1"], np.float64)
    Wqi = np.asarray(inputs["Wqi"], np.float64); bqi = np.asarray(inputs["bqi"], np.float64)
    Wki = np.asarray(inputs["Wki"], np.float64); bki = np.asarray(inputs["bki"], np.float64)
    Wg = np.asarray(inputs["Wg"], np.float64); bg = np.asarray(inputs["bg"], np.float64)
    Wp = np.asarray(inputs["Wp"], np.float64); bp = np.asarray(inputs["bp"], np.float64)
    ph = np.asarray(inputs["phase"], np.float64)

    u = Wki @ Wk1[:, 0]
    v = Wki @ bk1 + bki
    q = (x @ Wq.T + bq) @ Wqi.T + bqi                      # [B,S,A]
    qh = q.reshape(B, S, H, HD)
    alpha = np.einsum("bshe,he->bsh", qh, u.reshape(H, HD)) / np.sqrt(HD)
    beta = np.einsum("bshe,he->bsh", qh, v.reshape(H, HD)) / np.sqrt(HD)
    sc = alpha[..., None, :, None] * f[None, None, :, None, :] \
        + beta[..., None, :, None]                         # [B,S,D,H,F]
    sc -= sc.max(-1, keepdims=True)
    e = np.exp(sc)
    attn = e / e.sum(-1, keepdims=True)
    aw = attn.mean(-2)                                     # [B,S,D,F]
    t = np.linspace(0.0, 1.0, S)
    sig = TWO_PI * t[None, :, None, None] * f[None, None] + ph[None, None]
    ffs = np.sin(sig) * aw
    ffc = np.cos(sig) * aw
    ff = np.concatenate([ffs, ffc], axis=-1).reshape(B, S, D * 2 * F)
    ci = np.concatenate([x, ff], axis=-1)
    gate = 1.0 / (1.0 + np.exp(-(ci @ Wg.T + bg)))
    pp = ci @ Wp.T + bp
    silu = pp / (1.0 + np.exp(-pp))
    return (x + gate * silu).astype(np.float32)


def kernel(**inputs) -> np.ndarray:
    x = np.asarray(inputs["x"], np.float32)

    f = (np.asarray(inputs["freq_matrix"], np.float64)
         * np.asarray(inputs["freq_scale"], np.float64))
    if not np.all(f == f[0:1]):
        return _numpy_fallback(inputs)

    params, gsh, M_alpha, c_alpha = _fold_params(inputs)

    # exp-overflow guard (score = alpha*(g-gc); needs |score| < ~85)
    xmaxn = np.linalg.norm(x.reshape(-1, D), axis=1).max()
    amax = np.linalg.norm(M_alpha, axis=1).max() * xmaxn + np.abs(c_alpha).max()
    if amax * np.abs(gsh).max() > 85.0:
        return _numpy_fallback(inputs)

    key = "prog"
    if key not in _CACHE:
        _CACHE[key] = _build_program()
    nc = _CACHE[key]

    from concourse.bass_utils import run_bass_kernel_spmd

    in_maps = []
    for b in range(N_CORES):
        m = dict(params)
        m["xT"] = _make_xT(x[b])
        m["xn"] = np.ascontiguousarray(x[b])
        in_maps.append(m)

    res = run_bass_kernel_spmd(nc, in_maps, core_ids=list(range(N_CORES)))
    out = np.stack([res.results[b]["out"] for b in range(N_CORES)], axis=0)
    return out.astype(np.float32)


if __name__ == "__main__":
    import reference
    ins = {k: np.asarray(v) for k, v in reference.setup_inputs().items()}
    got = kernel(**ins)
    import jax.numpy as jnp
    exp = np.asarray(reference.reference(**{k: jnp.asarray(v) for k, v in ins.items()}))
    err = np.linalg.norm(got - exp) / np.linalg.norm(exp)
    print("rel err:", err)


# revision 72
# speedup vs baseline: 1.1452x; 1.1452x over previous
"""Trainium2 Bass kernel for nn_AdaptiveFourierFeatures.

Strategy
--------
The reference module computes, per batch b and token s:

    q[s,h,:]   depends on x[s] through two linear layers
    k[d,f,h,:] = f[d,f]*u[h,:] + v[h,:]         (keys are AFFINE in f[d,f]
                                                  because key_proj is Linear(1,A))
    scores[s,d,h,f] = q.k/sqrt(HD) = alpha[s,h]*f[d,f] + beta[s,h]

With the given inputs, freq_matrix*freq_scale has IDENTICAL rows
(f[d,:] == g[:] for all d), so softmax over f is d-independent and beta
cancels inside the softmax:

    attn[s,h,f] = softmax_f(alpha[s,h] * (g[f]-gc))      (gc: shift for range)
    aw[s,f]     = mean_h attn[s,h,f]

The fourier features contract with the MLP weights analytically using
sin(theta+phi) = sin*cos + cos*sin, folding phase and the D dimension into
small [F,O] matrices on the host.  The device pipeline per token is then:

    x(64) -> alpha-scores(64=H*F) -> softmax -> aw features z(32)
          -> [x|z|1](97) @ G(97x128) -> sigmoid*silu gate -> residual

Sharding: data-parallel over batch B=8, one batch element per NeuronCore.
All folded parameters are tiny and replicated.

kernel(**inputs) takes the FULL inputs and returns the FULL [B,S,D] output.
"""

import sys

import numpy as np
import ml_dtypes

# concourse (bass) lives in the trn repo; make sure it is importable even if
# the harness runs from a directory without the site defaults.
for _p in ("/opt/trn_rl_repo", "/opt/pypackages"):
    if _p not in sys.path:
        sys.path.append(_p)

# ---- problem constants (hardcoded; kernel.py must be self-contained) ----
B, S, D, F, A, H, O = 8, 2048, 64, 16, 32, 4, 64
HD = A // H
TWO_PI = 2.0 * np.pi
N_CORES = 8
HF = H * F            # 64 score columns per token
NFEAT = D + 2 * F + 1  # 97 = x | z_sin | z_cos | ones
HALF = S // 2          # stacked-half layout: 1024 tokens per half

BF16 = ml_dtypes.bfloat16
NCHUNKS = 2

_CACHE = {}


def _make_xT(xb: np.ndarray) -> np.ndarray:
    """[S, D] batch slice -> chunk-major transposed bf16 [D, S] layout."""
    xt = xb.T
    cw = HALF // NCHUNKS
    pieces = []
    for c in range(NCHUNKS):
        pieces.append(xt[:, c * cw:(c + 1) * cw])
        pieces.append(xt[:, HALF + c * cw:HALF + (c + 1) * cw])
    return np.ascontiguousarray(np.concatenate(pieces, axis=1)).astype(BF16)


def _build_program(nchunks: int = NCHUNKS, enable_asserts: bool = True,
                   enable_partition_id: bool = True,
                   ndum: int = 30, dumn: int = 128):
    """Build the 8-core SPMD bass program (per-core shapes)."""
    import concourse.bass as bass
    import concourse.bacc as bacc
    import concourse.tile as tile
    from concourse import mybir
    from bass_rust import add_dep_helper

    dt = mybir.dt
    AF = mybir.ActivationFunctionType
    ALU = mybir.AluOpType

    nc = bacc.Bacc("TRN2", target_bir_lowering=False, debug=False,
                   enable_asserts=enable_asserts, num_devices=N_CORES,
                   enable_partition_id=enable_partition_id)

    # ---- per-core DRAM parameters ----
    # all bf16 params packed into one [128, 361] array:
    #   wsc [64,64] @cols 0:64, o1 [128,8] @64:72, e2q [8,128] @72:200,
    #   o2 [128,32] @200:232, G [97,128] @232:360, b_score [128,1] @360
    xT = nc.dram_tensor("xT", [D, S], dt.bfloat16, kind="ExternalInput").ap()
    xn = nc.dram_tensor("xn", [S, D], dt.float32, kind="ExternalInput").ap()
    trig = nc.dram_tensor("trig", [2 * F, S], dt.bfloat16, kind="ExternalInput").ap()
    pk = nc.dram_tensor("pk", [128, 361], dt.bfloat16, kind="ExternalInput").ap()
    out_d = nc.dram_tensor("out", [S, D], dt.float32, kind="ExternalOutput").ap()

    KT = S // 128                # 16 token tiles of 128
    CW = HALF // nchunks         # stacked-column chunk width

    with tile.TileContext(nc) as tc:
        with (
            tc.tile_pool(name="const", bufs=1) as cpool,
            tc.tile_pool(name="sb", bufs=1) as sb,
            tc.tile_pool(name="work", bufs=3) as wk,
            tc.tile_pool(name="ps", bufs=3, space="PSUM") as ps,
            tc.tile_pool(name="ps_big", bufs=1, space="PSUM") as psb,
        ):
            # ---- inputs to SBUF (x first — it gates compute; two HWDGE rings) ----
            # CZ = [x^T (0:64) | zs (64:80) | zc (80:96) | ones (96)]
            # xT arrives chunk-major from the host: chunk c occupies source
            # columns [c*2CW, (c+1)*2CW) = tokens {c*CW..} U {HALF+c*CW..},
            # so chunk 0's scores can start after the first piece lands.
            cz = sb.tile([NFEAT, S], dt.bfloat16)
            czx_v = cz[0:D, :].rearrange("d (h c) -> d h c", h=2)
            for c in range(nchunks):
                lo = c * CW
                nc.sync.dma_start(out=czx_v[:, :, lo:lo + CW],
                                  in_=xT[:, c * 2 * CW:(c + 1) * 2 * CW])

            c_pk = cpool.tile([128, 361], dt.bfloat16)
            nc.scalar.dma_start(out=c_pk[:], in_=pk[:])
            c_wsc = c_pk[0:D, 0:64]
            c_o1 = c_pk[0:128, 64:72]
            c_e2q = c_pk[0:8, 72:200]
            c_o2 = c_pk[0:128, 200:232]
            c_g = c_pk[0:NFEAT, 232:360]

            # natural-layout x for the residual: [128, (k,64)].  Held back
            # until chunk 0's exp so its 512KB doesn't compete with the
            # critical x^T pieces on the SDMA engines (first use is ~19us).
            xn_t = sb.tile([128, KT * D], dt.float32)
            xn_dma = nc.scalar.dma_start(
                out=xn_t[:],
                in_=xn.rearrange("(k p) d -> p k d", p=128),
            )

            # trig table lives on partitions 64..95 to lane-align with CZ
            # (also held back past chunk 0's exp; first use ~15.6us)
            c_trig = cpool.tile([96, S], dt.bfloat16)
            trig_dma = nc.sync.dma_start(out=c_trig[64:96, :], in_=trig[:])

            # exp bias column (fp32 for the activation bias operand)
            c_bsc = cpool.tile([128, 1], dt.float32)
            nc.vector.tensor_copy(c_bsc[:], c_pk[:, 360:361])

            nc.vector.memset(cz[NFEAT - 1:NFEAT, :], 1.0)

            # warm up the activation table set (exp/tanh share one set)
            warm = cpool.tile([1, 2], dt.float32)
            nc.vector.memset(warm[:], 0.0)
            nc.scalar.activation(warm[:], warm[:], AF.Exp)

            # f32r copy of the 1/den-broadcast matrix for the f32r matmul
            e2f = cpool.tile([8, 128], dt.float32r)
            nc.vector.tensor_copy(e2f[:], c_e2q)

            # PE warm-up: full-array matmuls during the input-DMA wait flip
            # the HAM clock gate to 2.4 GHz before real work arrives.  The
            # operand tile is never written — garbage values are fine, the
            # psum slot is reused (and start=True-cleared) by later work.
            wgarb = cpool.tile([128, 512], dt.bfloat16)
            nc.gpsimd.memset(wgarb[:], 0.0)
            pdum = ps.tile([128, 512], dt.float32, tag="fr")
            for _ in range(ndum):
                nc.tensor.matmul(pdum[:, 0:dumn], wgarb[:, 0:128],
                                 wgarb[:, 0:dumn], tile_position=(0, 0))

            from concourse.dve_ops import (
                RECIP_APPROX_FAST_CONSTS as _RC,
                RECIPROCAL_APPROX_FAST as _RAF,
            )

            # front stages are emitted STAGE-MAJOR across chunks so chunk 1's
            # reciprocal sits ahead of chunk 0's attn-multiply in the DVE
            # FIFO and fills the wait for the rb broadcast matmul.  pdum/s2/
            # den/rb share one 3-slot psum tag (all 2KB; lifetimes interleave).
            ch = [dict(lo=c * CW, tok_los=(c * CW, HALF + c * CW))
                  for c in range(nchunks)]

            for c, st in enumerate(ch):
                # -- scores: S2[half*64+hf, col] = sum_d x^T[d, tok] Wsc[d, hf]
                s2 = ps.tile([128, CW], dt.float32, tag="fr")
                st["s2"] = s2
                for h in range(2):
                    t0 = st["tok_los"][h]
                    nc.tensor.matmul(
                        s2[h * 64:(h + 1) * 64, :], c_wsc,
                        cz[0:D, t0:t0 + CW], tile_position=(0, h * 64),
                    )
                # -- exp (bias adds the constant alpha-offset term)
                e1 = wk.tile([128, CW], dt.bfloat16, tag="e1")
                st["e1"] = e1
                exp_inst = nc.scalar.activation(e1[:], s2[:], AF.Exp,
                                                bias=c_bsc[:])
                if c == 0:
                    add_dep_helper(xn_dma.ins, exp_inst.ins, sync=True,
                                   reason="delay xn load past critical xT")

            for st in ch:
                # -- denominators: den[(half,h), col] = sum_f e1
                den = ps.tile([8, CW], dt.float32, tag="fr")
                nc.tensor.matmul(den[:], c_o1, st["e1"][:],
                                 tile_position=(0, 0))
                # -- reciprocal (fast Newton approx, ~18 bits), f32r-rounded
                rec = wk.tile([8, CW], dt.float32r, tag="rec")
                st["recip_inst"] = nc.vector._custom_dve(
                    _RAF, out=rec[:], in0=den[:],
                    s0=_RC["s0"], s1=_RC["s1"], imm2=_RC["imm2"])
                # -- broadcast 1/den back to all 128 rows (x0.25 head-mean);
                # f32r streams at bf16 rate for N>=256, no bf16 cast needed
                rb = ps.tile([128, CW], dt.float32, tag="fr")
                st["rb"] = rb
                nc.tensor.matmul(rb[:], e2f[:], rec[:], tile_position=(0, 0))

            for ci, st in enumerate(ch):
                # -- attn/4 = e1 * rb
                at = wk.tile([128, CW], dt.bfloat16, tag="at")
                st["at"] = at
                nc.vector.tensor_mul(at[:], st["e1"][:], st["rb"][:])

            for st in ch:
                lo = st["lo"]
                tok_los = st["tok_los"]
                at = st["at"]

                # -- aw rows (duplicated for sin/cos) on partitions 64..95
                aw = psb.tile([96, 2 * CW], dt.float32, tag="aw")
                for h in range(2):
                    for n0 in range(0, CW, 512):
                        nn = min(512, CW - n0)
                        nc.tensor.matmul(
                            aw[64:96, h * CW + n0:h * CW + n0 + nn],
                            c_o2[h * 64:(h + 1) * 64, :],
                            at[h * 64:(h + 1) * 64, n0:n0 + nn],
                            tile_position=(h * 64, 64),
                        )

                # -- per-half tail: z -> MLP -> tanh -> gate -> residual -> out
                kpc = CW // 128  # k-tiles per token range
                out_v = out_d.rearrange("(k p) d -> p k d", p=128)
                for h in range(2):
                    t0 = tok_los[h]
                    k0 = t0 // 128
                    # z features into CZ rows 64..96 (aw * sin/cos table)
                    nc.vector.tensor_mul(
                        cz[64:96, t0:t0 + CW],
                        aw[64:96, h * CW:(h + 1) * CW],
                        c_trig[64:96, t0:t0 + CW],
                    )
                    # MLP: per 128-token tile, pre = CZ_tile^T @ G (nat layout)
                    pre = psb.tile([128, kpc * 128], dt.float32, tag=f"pre{h}")
                    for i in range(kpc):
                        nc.tensor.matmul(
                            pre[:, i * 128:(i + 1) * 128],
                            cz[:, (k0 + i) * 128:(k0 + i + 1) * 128],
                            c_g,
                            tile_position=(0, 0),
                        )
                    pre_v = pre[:].rearrange("p (j o) -> p j o", j=kpc)
                    th = wk.tile([128, kpc * 128], dt.bfloat16, tag=f"th{h}")
                    th_v = th[:].rearrange("p (j o) -> p j o", j=kpc)
                    wt = wk.tile([128, kpc * 64], dt.bfloat16, tag=f"wt{h}")
                    wt_v = wt[:].rearrange("p (j o) -> p j o", j=kpc)
                    gt = wk.tile([128, kpc * 64], dt.bfloat16, tag=f"gt{h}")
                    out_t = wk.tile([128, kpc * 64], dt.float32, tag=f"outc{h}")
                    # tanh(pre/2); sigmoid(a)=0.5+0.5*tanh(a/2)
                    nc.scalar.activation(th[:], pre[:], AF.Tanh, scale=0.5)
                    # w = (1+tanh_p) * pre_p   [silu*2]
                    nc.vector.scalar_tensor_tensor(
                        wt_v, th_v[:, :, 64:128], 1.0, pre_v[:, :, 64:128],
                        ALU.add, ALU.mult,
                    )
                    # gated*4 = (1+tanh_g) * w
                    nc.vector.scalar_tensor_tensor(
                        gt[:], th_v[:, :, 0:64], 1.0, wt_v,
                        ALU.add, ALU.mult,
                    )
                    # out = gated*0.25 + x, then DMA out (HWDGE)
                    nc.vector.scalar_tensor_tensor(
                        out_t[:], gt[:], 0.25,
                        xn_t[:, k0 * 64:(k0 + kpc) * 64],
                        ALU.mult, ALU.add,
                    )
                    eng = nc.sync if h == 0 else nc.scalar
                    eng.dma_start(out=out_v[:, k0:k0 + kpc, :], in_=out_t[:])

    nc.compile()
    return nc


def _fold_params(inputs):
    """Host-side constant folding (float64).  Returns per-core arrays."""
    f = (np.asarray(inputs["freq_matrix"], np.float64)
         * np.asarray(inputs["freq_scale"], np.float64))
    g = f[0]
    gc = 0.5 * (g.max() + g.min())
    gsh = g - gc

    Wq = np.asarray(inputs["Wq"], np.float64)
    bq = np.asarray(inputs["bq"], np.float64)
    Wk1 = np.asarray(inputs["Wk1"], np.float64)
    bk1 = np.asarray(inputs["bk1"], np.float64)
    Wqi = np.asarray(inputs["Wqi"], np.float64)
    bqi = np.asarray(inputs["bqi"], np.float64)
    Wki = np.asarray(inputs["Wki"], np.float64)
    bki = np.asarray(inputs["bki"], np.float64)
    ph = np.asarray(inputs["phase"], np.float64)

    u = Wki @ Wk1[:, 0]
    Wqq = Wqi @ Wq
    bqq = Wqi @ bq + bqi
    u_h = u.reshape(H, HD)
    M_alpha = np.einsum("he,hed->hd", u_h, Wqq.reshape(H, HD, D)) / np.sqrt(HD)
    c_alpha = np.einsum("he,he->h", u_h, bqq.reshape(H, HD)) / np.sqrt(HD)

    W_score = np.einsum("hd,f->dhf", M_alpha, gsh).reshape(D, HF)
    b_score = np.einsum("h,f->hf", c_alpha, gsh).reshape(HF)
    b_score2 = np.concatenate([b_score, b_score]).reshape(128, 1)

    t = np.linspace(0.0, 1.0, S)
    theta = TWO_PI * t[:, None] * g[None, :]
    trig = np.concatenate([np.sin(theta).T, np.cos(theta).T], 0)  # [2F, S]

    cph, sph = np.cos(ph), np.sin(ph)

    def fold_mlp(W):
        W = np.asarray(W, np.float64)
        Wx = W[:, :D]
        Wf = W[:, D:].reshape(O, D, 2 * F)
        Ws, Wc = Wf[:, :, :F], Wf[:, :, F:]
        Us = np.einsum("df,odf->fo", cph, Ws) - np.einsum("df,odf->fo", sph, Wc)
        Uc = np.einsum("df,odf->fo", sph, Ws) + np.einsum("df,odf->fo", cph, Wc)
        return Wx, Us, Uc

    Wgx, Ugs, Ugc = fold_mlp(inputs["Wg"])
    Wpx, Ups, Upc = fold_mlp(inputs["Wp"])
    bg = np.asarray(inputs["bg"], np.float64)
    bp = np.asarray(inputs["bp"], np.float64)

    G = np.zeros((NFEAT, 128))
    G[0:D, 0:64] = Wgx.T
    G[D:D + F, 0:64] = Ugs
    G[D + F:D + 2 * F, 0:64] = Ugc
    G[NFEAT - 1, 0:64] = bg
    G[0:D, 64:128] = Wpx.T
    G[D:D + F, 64:128] = Ups
    G[D + F:D + 2 * F, 64:128] = Upc
    G[NFEAT - 1, 64:128] = bp

    # indicator matrices for the softmax plumbing
    p = np.arange(128)
    O1 = (p[:, None] // 16 == np.arange(8)[None, :]).astype(np.float64)
    E2q = 0.25 * (np.arange(8)[:, None] == p[None, :] // 16).astype(np.float64)
    O2 = ((p[:, None] % 16) == (np.arange(32)[None, :] % 16)).astype(np.float64)

    # pack all bf16 params into one [128, 361] array (see _build_program)
    pk = np.zeros((128, 361))
    pk[0:D, 0:64] = W_score
    pk[0:128, 64:72] = O1
    pk[0:8, 72:200] = E2q
    pk[0:128, 200:232] = O2
    pk[0:NFEAT, 232:360] = G
    pk[:, 360] = b_score2[:, 0]

    return dict(
        trig=trig.astype(BF16),
        pk=pk.astype(BF16),
    ), gsh, M_alpha, c_alpha


def _numpy_fallback(inputs):
    """Exact collapsed computation in numpy (general freq rows)."""
    x = np.asarray(inputs["x"], np.float64)
    f = (np.asarray(inputs["freq_matrix"], np.float64)
         * np.asarray(inputs["freq_scale"], np.float64))
    Wq = np.asarray(inputs["Wq"], np.float64); bq = np.asarray(inputs["bq"], np.float64)
    Wk1 = np.asarray(inputs["Wk1"], np.float64); bk1 = np.asarray(inputs["bk1"], np.float64)
    Wqi = np.asarray(inputs["Wqi"], np.float64); bqi = np.asarray(inputs["bqi"], np.float64)
    Wki = np.asarray(inputs["Wki"], np.float64); bki = np.asarray(inputs["bki"], np.float64)
    Wg = np.asarray(inputs["Wg"], np.float64); bg = np.asarray(inputs["bg"], np.float64)
    Wp = np.asarray(inputs["Wp"], np.float64); bp = np.asarray(inputs["bp"], np.float64)
    ph = np.asarray(inputs["phase"], np.float64)

    u = Wki @ Wk1[:, 0]
    v = Wki @ bk1 + bki
    q = (x @ Wq.T + bq) @ Wqi.T + bqi                      # [B,S,A]
    qh = q.reshape(B, S, H, HD)
    alpha = np.einsum("bshe,he->bsh", qh, u.reshape(H, HD)) / np.sqrt(HD)
    beta = np.einsum("bshe,he->bsh", qh, v.reshape(H, HD)) / np.sqrt(HD)
    sc = alpha[..., None, :, None] * f[None, None, :, None, :] \
        + beta[..., None, :, None]                         # [B,S,D,H,F]
    sc -= sc.max(-1, keepdims=True)
    e = np.exp(sc)
    attn = e / e.sum(-1, keepdims=True)
    aw = attn.mean(-2)                                     # [B,S,D,F]
    t = np.linspace(0.0, 1.0, S)
    sig = TWO_PI * t[None, :, None, None] * f[None, None] + ph[None, None]
    ffs = np.sin(sig) * aw
    ffc = np.cos(sig) * aw
    ff = np.concatenate([ffs, ffc], axis=-1).reshape(B, S, D * 2 * F)
    ci = np.concatenate([x, ff], axis=-1)
    gate = 1.0 / (1.0 + np.exp(-(ci @ Wg.T + bg)))
    pp = ci @ Wp.T + bp
    silu = pp / (1.0 + np.exp(-pp))
    return (x + gate * silu).astype(np.float32)


def kernel(**inputs) -> np.ndarray:
    x = np.asarray(inputs["x"], np.float32)

    f = (np.asarray(inputs["freq_matrix"], np.float64)
         * np.asarray(inputs["freq_scale"], np.float64))
    if not np.all(f == f[0:1]):
        return _numpy_fallback(inputs)

    params, gsh, M_alpha, c_alpha = _fold_params(inputs)

    # exp-overflow guard (score = alpha*(g-gc); needs |score| < ~85)
    xmaxn = np.linalg.norm(x.reshape(-1, D), axis=1).max()
    amax = np.linalg.norm(M_alpha, axis=1).max() * xmaxn + np.abs(c_alpha).max()
    if amax * np.abs(gsh).max() > 85.0:
        return _numpy_fallback(inputs)

    key = "prog"
    if key not in _CACHE:
        _CACHE[key] = _build_program()
    nc = _CACHE[key]

    from concourse.bass_utils import run_bass_kernel_spmd

    in_maps = []
    for b in range(N_CORES):
        m = dict(params)
        m["xT"] = _make_xT(x[b])
        m["xn"] = np.ascontiguousarray(x[b])
        in_maps.append(m)

    res = run_bass_kernel_spmd(nc, in_maps, core_ids=list(range(N_CORES)))
    out = np.stack([res.results[b]["out"] for b in range(N_CORES)], axis=0)
    return out.astype(np.float32)


if __name__ == "__main__":
    import reference
    ins = {k: np.asarray(v) for k, v in reference.setup_inputs().items()}
    got = kernel(**ins)
    import jax.numpy as jnp
    exp = np.asarray(reference.reference(**{k: jnp.asarray(v) for k, v in ins.items()}))
    err = np.linalg.norm(got - exp) / np.linalg.norm(exp)
    print("rel err:", err)
